# revision 2
# baseline (speedup 1.0000x reference)
"""Trainium2 Bass kernel for DeepSupervisionLoss (focal + boundary-weighted dice,
4 heads, deep supervision). Pure data-parallel over the batch dim across 8 cores;
each core reduces its shard to 16 partial scalars; host combines.

Math (per element, t binary, z = x*(2t-1)):
  v  = sigmoid(z) = pt            (ACT Sigmoid table)
  r  = ln(v) = -bce               (ACT Ln)
  u25 = (1-pt)^2.5 = exp(-2.5*(z - r))   (ACT Exp)
  focal head sum F = <u25, bce> = -<u25, r>
  dilate = maxpool3x3(t), erode = minpool3x3(t)  (cv2 borders: -inf / +inf)
  b = dilate - erode; W = 1 + 7b; wt = W*t; w0 = W - wt
  I = <v, wt>;  vw0 = <v, w0>;  SW = sum(W)
  dice = (2I+1) / (I - vw0 + SW + 1)   [since pred_sum+target_sum = I - vw0 + SW]
  head = 0.3*(0.25*F/N) + 0.7*(1-dice)
Dots <.,.> are diagonals of accumulated A^T B in PSUM (PE engine).
All bulk tensors are bf16 (casting SWDGE DMAs convert f32 inputs in flight);
sums accumulate in f32 (PSUM / accum_out).
"""
import sys

import numpy as np

for _p in ("/opt/trn_rl_repo",):
    if _p not in sys.path:
        sys.path.insert(0, _p)

import concourse.bacc as bacc  # noqa: E402
import concourse.mybir as mybir  # noqa: E402
from concourse import tile  # noqa: E402
from concourse.alu_op_type import AluOpType  # noqa: E402

F32 = mybir.dt.float32
BF16 = mybir.dt.bfloat16
AF = mybir.ActivationFunctionType

N_CORES = 8
N_IMG_TOTAL = 32
H = W = 512
P = 128              # partitions
RB = 4               # rows per partition
FD = RB * W          # 2048 free elems per image tile
NCH = 16             # 128-col chunks per tile
PRED_NAMES = ("main_pred", "ds1", "ds2", "ds3")

SIG_SET = "sigmoid_and_others"
NLE_SET = "natural_log_exp_and_others"


def build_nc(n_img):
    nc = bacc.Bacc("TRN2", target_bir_lowering=False, debug=False)

    xs = [nc.declare_dram_parameter(nm, [n_img, H, W], F32, isOutput=False)
          for nm in PRED_NAMES]
    tg = nc.declare_dram_parameter("target", [n_img, H, W], F32, isOutput=False)
    ident_d = nc.declare_dram_parameter("ident", [P, P], F32, isOutput=False)
    ones_d = nc.declare_dram_parameter("onescol", [P, 1], F32, isOutput=False)
    out_d = nc.declare_dram_parameter("out", [1, 16], F32, isOutput=True)

    def img_view(dram, i):
        # [512, 512] image -> [128, 2048]; partition p holds rows 4p..4p+3
        return dram.ap()[i].rearrange("(p a) w -> p (a w)", p=P)

    with tile.TileContext(nc) as tc:
        with (
            tc.tile_pool(name="consts", bufs=1) as cp,
            tc.tile_pool(name="tgt", bufs=2) as tp_,
            tc.tile_pool(name="hh", bufs=2) as hp,
            tc.tile_pool(name="mm", bufs=1) as mp,
            tc.tile_pool(name="dd", bufs=1) as dp,
            tc.tile_pool(name="ww", bufs=2) as wp,
            tc.tile_pool(name="xin", bufs=3) as xp,
            tc.tile_pool(name="zz", bufs=2) as zp,
            tc.tile_pool(name="vv", bufs=2) as vp,
            tc.tile_pool(name="rq", bufs=2) as rp,
            tc.tile_pool(name="psacc", bufs=1, space="PSUM") as pa,
            tc.tile_pool(name="pssh", bufs=1, space="PSUM") as ps,
        ):
            ident = cp.tile([P, P], F32)
            onescol = cp.tile([P, 1], F32)
            nc.sync.dma_start(out=ident[:], in_=ident_d.ap())
            nc.sync.dma_start(out=onescol[:], in_=ones_d.ap())

            acc16 = cp.tile([P, 16], F32)
            sWt = cp.tile([P, n_img], F32)
            nc.vector.memset(acc16[:], 0.0)

            # partition-shift buffers (bufs=1, persistent). Border rows hold
            # the pooling identity (0 for max/dilate, 1 for min/erode on
            # binary data) and are written only once; per-image DMAs fill the
            # other 127 rows.
            shup_x = cp.tile([P, W], BF16)
            shdn_x = cp.tile([P, W], BF16)
            shup_n = cp.tile([P, W], BF16)
            shdn_n = cp.tile([P, W], BF16)
            nc.vector.memset(shup_x[:], 0.0)
            nc.vector.memset(shdn_x[:], 0.0)
            nc.vector.memset(shup_n[:], 1.0)
            nc.vector.memset(shdn_n[:], 1.0)

            # PSUM accumulators: diag(A^T B) accumulation targets.
            # Pre-zeroed; all matmuls accumulate (start=False) so Tile's
            # PE reordering cannot race a start=True clear against earlier
            # contributions (order of pure accumulates is commutative).
            accIV = pa.tile([P, 8, P], F32)   # per pred: [I | vw0] (2 banks)
            accF = pa.tile([P, 4, P], F32)    # per pred: <u25, ln v> (1 bank)
            nc.vector.memset(accIV[:], 0.0)
            nc.vector.memset(accF[:], 0.0)

            for img in range(n_img):
                # ---------------- target pipeline ----------------
                tb = tp_.tile([P, RB, W], BF16, name="tb")
                # casting SWDGE: f32 DRAM -> bf16 SBUF in flight
                nc.gpsimd.dma_start(out=tb[:], in_=img_view(tg, img).rearrange(
                    "p (a w) -> p a w", a=RB))

                msign = tp_.tile([P, FD], BF16, name="msign")     # 2t-1
                nc.vector.tensor_scalar(
                    out=msign[:], in0=tb[:].rearrange("p a w -> p (a w)"),
                    scalar1=2.0, scalar2=-1.0, op0=AluOpType.mult, op1=AluOpType.add)

                # horizontal 3-tap max (dilate) / min (erode); cv2 border
                # excluded at the edges (identity element of the op)
                Ax = hp.tile([P, RB, W], BF16, name="Ax")
                An = hp.tile([P, RB, W], BF16, name="An")
                hx = hp.tile([P, RB, W], BF16, name="hx")
                hn = hp.tile([P, RB, W], BF16, name="hn")
                nc.vector.tensor_tensor(out=Ax[:, :, 0:W - 2], in0=tb[:, :, 0:W - 2],
                                        in1=tb[:, :, 2:W], op=AluOpType.max)
                nc.vector.tensor_tensor(out=hx[:, :, 1:W - 1], in0=Ax[:, :, 0:W - 2],
                                        in1=tb[:, :, 1:W - 1], op=AluOpType.max)
                nc.vector.tensor_tensor(out=hx[:, :, 0:1], in0=tb[:, :, 0:1],
                                        in1=tb[:, :, 1:2], op=AluOpType.max)
                nc.vector.tensor_tensor(out=hx[:, :, W - 1:W], in0=tb[:, :, W - 2:W - 1],
                                        in1=tb[:, :, W - 1:W], op=AluOpType.max)
                nc.vector.tensor_tensor(out=An[:, :, 0:W - 2], in0=tb[:, :, 0:W - 2],
                                        in1=tb[:, :, 2:W], op=AluOpType.min)
                nc.vector.tensor_tensor(out=hn[:, :, 1:W - 1], in0=An[:, :, 0:W - 2],
                                        in1=tb[:, :, 1:W - 1], op=AluOpType.min)
                nc.vector.tensor_tensor(out=hn[:, :, 0:1], in0=tb[:, :, 0:1],
                                        in1=tb[:, :, 1:2], op=AluOpType.min)
                nc.vector.tensor_tensor(out=hn[:, :, W - 1:W], in0=tb[:, :, W - 2:W - 1],
                                        in1=tb[:, :, W - 1:W], op=AluOpType.min)

                # cross-partition neighbor rows via SBUF->SBUF DMA shifts
                nc.sync.dma_start(out=shup_x[1:P, :], in_=hx[0:P - 1, 3, :])
                nc.sync.dma_start(out=shdn_x[0:P - 1, :], in_=hx[1:P, 0, :])
                nc.sync.dma_start(out=shup_n[1:P, :], in_=hn[0:P - 1, 3, :])
                nc.sync.dma_start(out=shdn_n[0:P - 1, :], in_=hn[1:P, 0, :])

                # vertical 3-tap max/min within/across partitions
                Mx = mp.tile([P, 3, W], BF16, name="Mx")
                Mn = mp.tile([P, 3, W], BF16, name="Mn")
                Dx = dp.tile([P, RB, W], BF16, name="Dx")   # dilated
                En = dp.tile([P, RB, W], BF16, name="En")   # eroded
                nc.vector.tensor_tensor(out=Mx[:], in0=hx[:, 0:3, :],
                                        in1=hx[:, 1:4, :], op=AluOpType.max)
                nc.vector.tensor_tensor(out=Dx[:, 1:3, :], in0=Mx[:, 0:2, :],
                                        in1=hx[:, 2:4, :], op=AluOpType.max)
                nc.vector.tensor_tensor(out=Dx[:, 0, :], in0=Mx[:, 0, :],
                                        in1=shup_x[:], op=AluOpType.max)
                nc.vector.tensor_tensor(out=Dx[:, 3, :], in0=Mx[:, 2, :],
                                        in1=shdn_x[:], op=AluOpType.max)
                nc.vector.tensor_tensor(out=Mn[:], in0=hn[:, 0:3, :],
                                        in1=hn[:, 1:4, :], op=AluOpType.min)
                nc.vector.tensor_tensor(out=En[:, 1:3, :], in0=Mn[:, 0:2, :],
                                        in1=hn[:, 2:4, :], op=AluOpType.min)
                nc.vector.tensor_tensor(out=En[:, 0, :], in0=Mn[:, 0, :],
                                        in1=shup_n[:], op=AluOpType.min)
                nc.vector.tensor_tensor(out=En[:, 3, :], in0=Mn[:, 2, :],
                                        in1=shdn_n[:], op=AluOpType.min)

                # b = Dx - En ;  W = 1 + 7b (sum accumulated) ; wt = W*t ; w0 = W - wt
                bbs = wp.tile([P, FD], BF16, name="bbs")
                nc.vector.tensor_tensor(
                    out=bbs[:], in0=Dx[:].rearrange("p a w -> p (a w)"),
                    in1=En[:].rearrange("p a w -> p (a w)"), op=AluOpType.subtract)
                Wt_ = wp.tile([P, FD], BF16, name="Wt_")
                nc.vector.tensor_scalar(
                    out=Wt_[:], in0=bbs[:], scalar1=7.0, scalar2=1.0,
                    op0=AluOpType.mult, op1=AluOpType.add,
                    accum_out=sWt[:, img:img + 1])
                WW = wp.tile([P, 2, FD], BF16, name="WW")
                nc.vector.tensor_tensor(
                    out=WW[:, 0, :], in0=Wt_[:],
                    in1=tb[:].rearrange("p a w -> p (a w)"), op=AluOpType.mult)
                nc.vector.tensor_tensor(
                    out=WW[:, 1, :], in0=Wt_[:], in1=WW[:, 0, :],
                    op=AluOpType.subtract)

                # ---------------- pred pipeline (4 heads, head pairs) ------
                last = img == n_img - 1
                for j in range(2):
                    z2 = zp.tile([P, 2, FD], BF16, name="z2")
                    for i in range(2):
                        x_t = xp.tile([P, FD], BF16, name="x_t")
                        # casting SWDGE load (f32 -> bf16); SWDGE also has
                        # enough wait slots for the WAR deps here.
                        nc.gpsimd.dma_start(out=x_t[:], in_=img_view(xs[2 * j + i], img))
                        nc.vector.tensor_tensor(out=z2[:, i, :], in0=x_t[:],
                                                in1=msign[:], op=AluOpType.mult)

                    v2 = vp.tile([P, 2, FD], BF16, name="v2")
                    nc.scalar.activation(v2[:], z2[:], AF.Sigmoid)
                    r2 = rp.tile([P, 2, FD], BF16, name="r2")
                    nc.scalar.activation(r2[:], v2[:], AF.Ln)
                    q2 = rp.tile([P, 2, FD], BF16, name="q2")
                    nc.vector.tensor_tensor(out=q2[:], in0=z2[:], in1=r2[:],
                                            op=AluOpType.subtract)
                    u2 = rp.tile([P, 2, FD], BF16, name="u2")
                    nc.scalar.activation(u2[:], q2[:], AF.Exp, scale=-2.5)

                    for i in range(2):
                        k = 2 * j + i
                        for c in range(NCH):
                            cs = slice(c * P, (c + 1) * P)
                            nc.tensor.matmul(
                                accIV[:, 2 * k:2 * k + 2, :],
                                v2[:, i, cs], WW[:, :, cs],
                                start=False, stop=(last and c == NCH - 1),
                                skip_group_check=True)
                            nc.tensor.matmul(
                                accF[:, k, :],
                                u2[:, i, cs], r2[:, i, cs],
                                start=False, stop=(last and c == NCH - 1),
                                skip_group_check=True)

            # ---------------- final reduction ----------------
            nc.vector.tensor_reduce(out=acc16[:, 12:13], in_=sWt[:],
                                    axis=mybir.AxisListType.X, op=AluOpType.add)
            dscr = cp.tile([P, P], F32)
            for j in range(8):
                nc.vector.scalar_tensor_tensor(
                    out=dscr[:], in0=accIV[:, j, :], scalar=1.0, in1=ident[:],
                    op0=AluOpType.mult, op1=AluOpType.mult,
                    accum_out=acc16[:, j:j + 1])
            for j in range(4):
                nc.vector.scalar_tensor_tensor(
                    out=dscr[:], in0=accF[:, j, :], scalar=1.0, in1=ident[:],
                    op0=AluOpType.mult, op1=AluOpType.mult,
                    accum_out=acc16[:, 8 + j:9 + j])

            fin = ps.tile([1, 16], F32, name="fin")
            nc.tensor.matmul(fin[:], onescol[:], acc16[:], start=True, stop=True)
            out_sb = cp.tile([1, 16], F32)
            nc.vector.tensor_copy(out=out_sb[:], in_=fin[:])
            nc.sync.dma_start(out=out_d.ap(), in_=out_sb[:])

    _pin_act_tables(nc)
    nc.finalize()
    return nc


def _pin_act_tables(nc):
    """The stock insertion pass places an InstLoadActFuncSet wherever the
    required table may be missing, but picks minimal sets (e.g. natural_log
    without exp), causing extra reloads. Post-process: walk the final ACT
    program order, retarget every load to the widest set covering the funcs
    up to the next load (sigmoid_and_others / natural_log_exp_and_others),
    and drop loads that keep the set unchanged."""
    orig = nc.insert_act_table_loads

    _WILD = {AF.Copy, AF.Identity, AF.MemsetZero, AF.Abs, AF.Sign, AF.Square,
             AF.Relu}

    def patched():
        orig()
        from concourse.hw_specs import get_activation_tables
        tables = get_activation_tables(nc.m.arch)
        names = list(tables.keys())
        sig_id, nle_id = names.index(SIG_SET), names.index(NLE_SET)

        def set_for(funcs):
            f = funcs - _WILD
            if not f:
                return None
            if f <= {AF.Sigmoid}:
                return sig_id
            if f <= {AF.Ln, AF.Exp}:
                return nle_id
            raise AssertionError(f"act funcs {f} not coverable by one table set")

        for fn in nc.m.functions:
            for blk in fn.blocks:
                insts = blk.instructions
                # segments: load index -> funcs used until the next load
                loads = [i for i, ins in enumerate(insts)
                         if isinstance(ins, mybir.InstLoadActFuncSet)]
                if not loads:
                    continue
                drop = set()
                cur = None
                for li, idx in enumerate(loads):
                    end = loads[li + 1] if li + 1 < len(loads) else len(insts)
                    funcs = {ins.func for ins in insts[idx:end]
                             if isinstance(ins, mybir.InstActivation)}
                    req = set_for(funcs)
                    if req is None or req == cur:
                        drop.add(idx)
                        continue
                    insts[idx].act_func_set_id = req
                    cur = req
                if drop:
                    blk.instructions[:] = [
                        ins for i, ins in enumerate(insts) if i not in drop]
                # safety: every activation must be covered by the live set
                cur = None
                for ins in blk.instructions:
                    if isinstance(ins, mybir.InstLoadActFuncSet):
                        cur = ins.act_func_set_id
                    elif isinstance(ins, mybir.InstActivation):
                        if ins.func in _WILD:
                            continue
                        assert cur is not None and ins.func in tables[names[cur]], (
                            f"{ins.func} not in table set {names[cur] if cur is not None else None}")

    nc.insert_act_table_loads = patched


def _consts():
    ident = np.eye(P, dtype=np.float32)
    ones = np.ones((P, 1), dtype=np.float32)
    return {"ident": ident, "onescol": ones}


_NC_CACHE = {}


def _get_nc(n_img):
    if n_img not in _NC_CACHE:
        _NC_CACHE[n_img] = build_nc(n_img)
    return _NC_CACHE[n_img]


def combine_partials(outs, n_total_elems):
    """outs: list of [1,16] f32 per core -> final scalar (float64 host math).
    acc16 layout: col 2k = I_k, col 2k+1 = vw0_k, col 8+k = <u25, ln v>_k,
    col 12 = sum(W)."""
    s = np.zeros(16, dtype=np.float64)
    for o in outs:
        s += np.asarray(o, dtype=np.float64).reshape(16)
    SW = s[12]
    total = 0.0
    for k, c in enumerate((1.0, 0.4, 0.2, 0.1)):
        I = s[2 * k]
        vw0 = s[2 * k + 1]
        F = -s[8 + k]                      # <u25, bce> = -<u25, ln v>
        f = 0.25 * F / n_total_elems
        dice = (2.0 * I + 1.0) / (I - vw0 + SW + 1.0)
        total += c * (0.3 * f + 0.7 * (1.0 - dice))
    return np.float32(total)


def kernel(main_pred, ds1, ds2, ds3, target, _trace=False):
    from concourse.bass_utils import run_bass_kernel_spmd

    n_img = N_IMG_TOTAL // N_CORES
    nc = _get_nc(n_img)
    consts = _consts()
    preds = {"main_pred": main_pred, "ds1": ds1, "ds2": ds2, "ds3": ds3}
    in_maps = []
    for core in range(N_CORES):
        sl = slice(core * n_img, (core + 1) * n_img)
        m = {nm: np.ascontiguousarray(
                np.asarray(v).reshape(N_IMG_TOTAL, H, W)[sl]).astype(np.float32)
             for nm, v in preds.items()}
        m["target"] = np.ascontiguousarray(
            np.asarray(target).reshape(N_IMG_TOTAL, H, W)[sl]).astype(np.float32)
        m.update(consts)
        in_maps.append(m)

    res = run_bass_kernel_spmd(nc, in_maps, list(range(N_CORES)), trace=_trace)
    outs = [r["out"] for r in res.results]
    total = combine_partials(outs, N_IMG_TOTAL * H * W)
    if _trace:
        kernel._last_result = res
    return np.asarray(total, dtype=np.float32)


# revision 7
# speedup vs baseline: 1.5864x; 1.5864x over previous
"""Trainium2 Bass kernel for DeepSupervisionLoss (focal + boundary-weighted dice,
4 heads, deep supervision). Pure data-parallel over the batch dim across 8 cores;
each core reduces its shard to 16 partial scalars; host combines.

Math (per element, t binary, z = x*(2t-1)):
  v  = sigmoid(z) = pt            (ACT Sigmoid table)
  r  = ln(v) = -bce               (ACT Ln)
  u25 = (1-pt)^2.5 = exp(-2.5*(z - r))   (ACT Exp)
  focal head sum F = <u25, bce> = -<u25, r>
  dilate = maxpool3x3(t), erode = minpool3x3(t)  (cv2 borders: -inf / +inf)
  b = dilate - erode; W = 1 + 7b; wt = W*t; w0 = W - wt
  I = <v, wt>;  vw0 = <v, w0>;  SW = sum(W)
  dice = (2I+1) / (I - vw0 + SW + 1)   [since pred_sum+target_sum = I - vw0 + SW]
  head = 0.3*(0.25*F/N) + 0.7*(1-dice)
Dots <.,.> are diagonals of accumulated A^T B in PSUM (PE engine).
All bulk tensors are bf16 (casting SWDGE DMAs convert f32 inputs in flight);
sums accumulate in f32 (PSUM / accum_out).
"""
import sys

import numpy as np

for _p in ("/opt/trn_rl_repo",):
    if _p not in sys.path:
        sys.path.insert(0, _p)

import concourse.bacc as bacc  # noqa: E402
import concourse.mybir as mybir  # noqa: E402
from concourse import tile  # noqa: E402
from concourse.alu_op_type import AluOpType  # noqa: E402

F32 = mybir.dt.float32
BF16 = mybir.dt.bfloat16
AF = mybir.ActivationFunctionType

N_CORES = 8
N_IMG_TOTAL = 32
H = W = 512
P = 128              # partitions
RB = 4               # rows per partition
FD = RB * W          # 2048 free elems per image tile
NCH = 16             # 128-col chunks per tile
PRED_NAMES = ("main_pred", "ds1", "ds2", "ds3")

SIG_SET = "sigmoid_and_others"
NLE_SET = "natural_log_exp_and_others"


def build_nc(n_img):
    nc = bacc.Bacc("TRN2", target_bir_lowering=False, debug=False)

    xs = [nc.declare_dram_parameter(nm, [n_img, H, W], F32, isOutput=False)
          for nm in PRED_NAMES]
    tg = nc.declare_dram_parameter("target", [n_img, H, W], F32, isOutput=False)
    wup_d = nc.declare_dram_parameter("wup", [P, P], BF16, isOutput=False)
    wdn_d = nc.declare_dram_parameter("wdn", [P, P], BF16, isOutput=False)
    ident_d = nc.declare_dram_parameter("ident", [P, P], F32, isOutput=False)
    ones_d = nc.declare_dram_parameter("onescol", [P, 1], F32, isOutput=False)
    out_d = nc.declare_dram_parameter("out", [1, 16], F32, isOutput=True)

    def img_view(dram, i):
        # [512, 512] image -> [128, 2048]; partition p holds rows 4p..4p+3
        return dram.ap()[i].rearrange("(p a) w -> p (a w)", p=P)

    with tile.TileContext(nc) as tc:
        with (
            tc.tile_pool(name="consts", bufs=1) as cp,
            tc.tile_pool(name="tgt", bufs=2) as tp_,
            tc.tile_pool(name="aa", bufs=1) as ap_,
            tc.tile_pool(name="hh", bufs=2) as hp,
            tc.tile_pool(name="mm", bufs=2) as mp,
            tc.tile_pool(name="dd", bufs=2) as dp,
            tc.tile_pool(name="ww", bufs=2) as wp,
            tc.tile_pool(name="xin", bufs=3) as xp,
            tc.tile_pool(name="zz", bufs=2) as zp,
            tc.tile_pool(name="vv", bufs=2) as vp,
            tc.tile_pool(name="rq", bufs=2) as rp,
            tc.tile_pool(name="psacc", bufs=1, space="PSUM") as pa,
            tc.tile_pool(name="pssh", bufs=1, space="PSUM") as ps,
        ):
            wup = cp.tile([P, P], BF16)
            wdn = cp.tile([P, P], BF16)
            ident = cp.tile([P, P], F32)
            onescol = cp.tile([P, 1], F32)
            nc.sync.dma_start(out=wup[:], in_=wup_d.ap())
            nc.sync.dma_start(out=wdn[:], in_=wdn_d.ap())
            nc.sync.dma_start(out=ident[:], in_=ident_d.ap())
            nc.sync.dma_start(out=onescol[:], in_=ones_d.ap())

            acc16 = cp.tile([P, 16], F32)
            sWt = cp.tile([P, n_img], F32)
            nc.vector.memset(acc16[:], 0.0)

            # PSUM accumulators: diag(A^T B) accumulation targets.
            # Pre-zeroed; all matmuls accumulate (start=False) so Tile's
            # PE reordering cannot race a start=True clear against earlier
            # contributions (order of pure accumulates is commutative).
            accIV = pa.tile([P, 8, P], F32)   # per pred: [I | vw0] (2 banks)
            accF = pa.tile([P, 4, P], F32)    # per pred: <u25, ln v> (1 bank)
            nc.vector.memset(accIV[:], 0.0)
            nc.vector.memset(accF[:], 0.0)

            for img in range(n_img):
                # ---------------- target pipeline ----------------
                tb = tp_.tile([P, RB, W], BF16, name="tb")
                # casting SWDGE: f32 DRAM -> bf16 SBUF in flight
                nc.gpsimd.dma_start(out=tb[:], in_=img_view(tg, img).rearrange(
                    "p (a w) -> p a w", a=RB))

                msign = tp_.tile([P, FD], BF16, name="msign")     # 2t-1
                nc.vector.tensor_scalar(
                    out=msign[:], in0=tb[:].rearrange("p a w -> p (a w)"),
                    scalar1=2.0, scalar2=-1.0, op0=AluOpType.mult, op1=AluOpType.add)
                tp = tp_.tile([P, RB, W], BF16, name="tp")        # 1-t
                nc.vector.tensor_scalar(
                    out=tp[:], in0=tb[:], scalar1=-1.0, scalar2=1.0,
                    op0=AluOpType.mult, op1=AluOpType.add)

                # horizontal 3-tap max of t (dilate) and of 1-t (erode
                # complement); zero pad at image edges is exact for both.
                Ax = ap_.tile([P, RB, W], BF16, name="Ax")
                An = ap_.tile([P, RB, W], BF16, name="An")
                hx = hp.tile([P, RB, W], BF16, name="hx")
                hn = hp.tile([P, RB, W], BF16, name="hn")
                for (src, A, h) in ((tb, Ax, hx), (tp, An, hn)):
                    nc.vector.tensor_tensor(out=A[:, :, 0:W - 2], in0=src[:, :, 0:W - 2],
                                            in1=src[:, :, 2:W], op=AluOpType.max)
                    nc.vector.tensor_tensor(out=h[:, :, 1:W - 1], in0=A[:, :, 0:W - 2],
                                            in1=src[:, :, 1:W - 1], op=AluOpType.max)
                    nc.vector.tensor_tensor(out=h[:, :, 0:1], in0=src[:, :, 0:1],
                                            in1=src[:, :, 1:2], op=AluOpType.max)
                    nc.vector.tensor_tensor(out=h[:, :, W - 1:W], in0=src[:, :, W - 2:W - 1],
                                            in1=src[:, :, W - 1:W], op=AluOpType.max)

                # cross-partition neighbor rows via PE shift matmuls (zero-fill
                # rows, exact for max of non-negative data)
                shx = ps.tile([P, 2, W], F32, name="shx")
                shn = ps.tile([P, 2, W], F32, name="shn")
                nc.tensor.matmul(shx[:, 0, :], wup[:], hx[:, 3, :], start=True, stop=True)
                nc.tensor.matmul(shx[:, 1, :], wdn[:], hx[:, 0, :], start=True, stop=True)
                nc.tensor.matmul(shn[:, 0, :], wup[:], hn[:, 3, :], start=True, stop=True)
                nc.tensor.matmul(shn[:, 1, :], wdn[:], hn[:, 0, :], start=True, stop=True)

                # vertical 3-tap max within/across partitions
                Mx = mp.tile([P, 3, W], BF16, name="Mx")
                Mn = mp.tile([P, 3, W], BF16, name="Mn")
                Dx = dp.tile([P, RB, W], BF16, name="Dx")   # dilate(t)
                Dn = dp.tile([P, RB, W], BF16, name="Dn")   # dilate(1-t)
                for (h, M, D, sh) in ((hx, Mx, Dx, shx), (hn, Mn, Dn, shn)):
                    nc.vector.tensor_tensor(out=M[:], in0=h[:, 0:3, :],
                                            in1=h[:, 1:4, :], op=AluOpType.max)
                    nc.vector.tensor_tensor(out=D[:, 1:3, :], in0=M[:, 0:2, :],
                                            in1=h[:, 2:4, :], op=AluOpType.max)
                    nc.vector.tensor_tensor(out=D[:, 0, :], in0=M[:, 0, :],
                                            in1=sh[:, 0, :], op=AluOpType.max)
                    nc.vector.tensor_tensor(out=D[:, 3, :], in0=M[:, 2, :],
                                            in1=sh[:, 1, :], op=AluOpType.max)

                # b = Dx + Dn - 1 ;  W = 1 + 7b = 7(Dx+Dn) - 6 (sum accum'd) ;
                # wt = W*t ; w0 = W - wt
                bbs = wp.tile([P, FD], BF16, name="bbs")
                nc.vector.tensor_tensor(
                    out=bbs[:], in0=Dx[:].rearrange("p a w -> p (a w)"),
                    in1=Dn[:].rearrange("p a w -> p (a w)"), op=AluOpType.add)
                Wt_ = wp.tile([P, FD], BF16, name="Wt_")
                nc.vector.tensor_scalar(
                    out=Wt_[:], in0=bbs[:], scalar1=7.0, scalar2=-6.0,
                    op0=AluOpType.mult, op1=AluOpType.add,
                    accum_out=sWt[:, img:img + 1])
                WW = wp.tile([P, 2, FD], BF16, name="WW")
                nc.vector.tensor_tensor(
                    out=WW[:, 0, :], in0=Wt_[:],
                    in1=tb[:].rearrange("p a w -> p (a w)"), op=AluOpType.mult)
                nc.vector.tensor_tensor(
                    out=WW[:, 1, :], in0=Wt_[:], in1=WW[:, 0, :],
                    op=AluOpType.subtract)

                # ---------------- pred pipeline (4 heads, head pairs) ------
                last = img == n_img - 1
                for j in range(2):
                    z2 = zp.tile([P, 2, FD], BF16, name="z2")
                    for i in range(2):
                        x_t = xp.tile([P, FD], BF16, name="x_t")
                        # casting SWDGE load (f32 -> bf16); SWDGE also has
                        # enough wait slots for the WAR deps here.
                        nc.gpsimd.dma_start(out=x_t[:], in_=img_view(xs[2 * j + i], img))
                        nc.vector.tensor_tensor(out=z2[:, i, :], in0=x_t[:],
                                                in1=msign[:], op=AluOpType.mult)

                    v2 = vp.tile([P, 2, FD], BF16, name="v2")
                    nc.scalar.activation(v2[:], z2[:], AF.Sigmoid)
                    r2 = rp.tile([P, 2, FD], BF16, name="r2")
                    nc.scalar.activation(r2[:], v2[:], AF.Ln)
                    q2 = rp.tile([P, 2, FD], BF16, name="q2")
                    nc.vector.tensor_tensor(out=q2[:], in0=z2[:], in1=r2[:],
                                            op=AluOpType.subtract)
                    u2 = rp.tile([P, 2, FD], BF16, name="u2")
                    nc.scalar.activation(u2[:], q2[:], AF.Exp, scale=-2.5)

                    for i in range(2):
                        k = 2 * j + i
                        for c in range(NCH):
                            cs = slice(c * P, (c + 1) * P)
                            nc.tensor.matmul(
                                accIV[:, 2 * k:2 * k + 2, :],
                                v2[:, i, cs], WW[:, :, cs],
                                start=False, stop=(last and c == NCH - 1),
                                skip_group_check=True)
                            nc.tensor.matmul(
                                accF[:, k, :],
                                u2[:, i, cs], r2[:, i, cs],
                                start=False, stop=(last and c == NCH - 1),
                                skip_group_check=True)

            # ---------------- final reduction ----------------
            nc.vector.tensor_reduce(out=acc16[:, 12:13], in_=sWt[:],
                                    axis=mybir.AxisListType.X, op=AluOpType.add)
            dscr = cp.tile([P, P], F32)
            for j in range(8):
                nc.vector.scalar_tensor_tensor(
                    out=dscr[:], in0=accIV[:, j, :], scalar=1.0, in1=ident[:],
                    op0=AluOpType.mult, op1=AluOpType.mult,
                    accum_out=acc16[:, j:j + 1])
            for j in range(4):
                nc.vector.scalar_tensor_tensor(
                    out=dscr[:], in0=accF[:, j, :], scalar=1.0, in1=ident[:],
                    op0=AluOpType.mult, op1=AluOpType.mult,
                    accum_out=acc16[:, 8 + j:9 + j])

            fin = ps.tile([1, 16], F32, name="fin")
            nc.tensor.matmul(fin[:], onescol[:], acc16[:], start=True, stop=True)
            out_sb = cp.tile([1, 16], F32)
            nc.vector.tensor_copy(out=out_sb[:], in_=fin[:])
            nc.sync.dma_start(out=out_d.ap(), in_=out_sb[:])

    _pin_act_tables(nc)
    nc.finalize()
    return nc


def _pin_act_tables(nc):
    """The stock insertion pass places an InstLoadActFuncSet wherever the
    required table may be missing, but picks minimal sets (e.g. natural_log
    without exp), causing extra reloads. Post-process: walk the final ACT
    program order, retarget every load to the widest set covering the funcs
    up to the next load (sigmoid_and_others / natural_log_exp_and_others),
    and drop loads that keep the set unchanged."""
    orig = nc.insert_act_table_loads

    _WILD = {AF.Copy, AF.Identity, AF.MemsetZero, AF.Abs, AF.Sign, AF.Square,
             AF.Relu}

    def patched():
        orig()
        from concourse.hw_specs import get_activation_tables
        tables = get_activation_tables(nc.m.arch)
        names = list(tables.keys())
        sig_id, nle_id = names.index(SIG_SET), names.index(NLE_SET)

        def set_for(funcs):
            f = funcs - _WILD
            if not f:
                return None
            if f <= {AF.Sigmoid}:
                return sig_id
            if f <= {AF.Ln, AF.Exp}:
                return nle_id
            raise AssertionError(f"act funcs {f} not coverable by one table set")

        for fn in nc.m.functions:
            for blk in fn.blocks:
                insts = blk.instructions
                # segments: load index -> funcs used until the next load
                loads = [i for i, ins in enumerate(insts)
                         if isinstance(ins, mybir.InstLoadActFuncSet)]
                if not loads:
                    continue
                drop = set()
                cur = None
                for li, idx in enumerate(loads):
                    end = loads[li + 1] if li + 1 < len(loads) else len(insts)
                    funcs = {ins.func for ins in insts[idx:end]
                             if isinstance(ins, mybir.InstActivation)}
                    req = set_for(funcs)
                    if req is None or req == cur:
                        drop.add(idx)
                        continue
                    insts[idx].act_func_set_id = req
                    cur = req
                if drop:
                    blk.instructions[:] = [
                        ins for i, ins in enumerate(insts) if i not in drop]
                # safety: every activation must be covered by the live set
                cur = None
                for ins in blk.instructions:
                    if isinstance(ins, mybir.InstLoadActFuncSet):
                        cur = ins.act_func_set_id
                    elif isinstance(ins, mybir.InstActivation):
                        if ins.func in _WILD:
                            continue
                        assert cur is not None and ins.func in tables[names[cur]], (
                            f"{ins.func} not in table set {names[cur] if cur is not None else None}")

    nc.insert_act_table_loads = patched


def _consts():
    import ml_dtypes
    wup = np.eye(P, k=1).astype(ml_dtypes.bfloat16)   # out[p] = in[p-1], 0 at p=0
    wdn = np.eye(P, k=-1).astype(ml_dtypes.bfloat16)  # out[p] = in[p+1], 0 at p=127
    ident = np.eye(P, dtype=np.float32)
    ones = np.ones((P, 1), dtype=np.float32)
    return {"wup": wup, "wdn": wdn, "ident": ident, "onescol": ones}


_NC_CACHE = {}


def _get_nc(n_img):
    if n_img not in _NC_CACHE:
        _NC_CACHE[n_img] = build_nc(n_img)
    return _NC_CACHE[n_img]


def combine_partials(outs, n_total_elems):
    """outs: list of [1,16] f32 per core -> final scalar (float64 host math).
    acc16 layout: col 2k = I_k, col 2k+1 = vw0_k, col 8+k = <u25, ln v>_k,
    col 12 = sum(W)."""
    s = np.zeros(16, dtype=np.float64)
    for o in outs:
        s += np.asarray(o, dtype=np.float64).reshape(16)
    SW = s[12]
    total = 0.0
    for k, c in enumerate((1.0, 0.4, 0.2, 0.1)):
        I = s[2 * k]
        vw0 = s[2 * k + 1]
        F = -s[8 + k]                      # <u25, bce> = -<u25, ln v>
        f = 0.25 * F / n_total_elems
        dice = (2.0 * I + 1.0) / (I - vw0 + SW + 1.0)
        total += c * (0.3 * f + 0.7 * (1.0 - dice))
    return np.float32(total)


def kernel(main_pred, ds1, ds2, ds3, target, _trace=False):
    from concourse.bass_utils import run_bass_kernel_spmd

    n_img = N_IMG_TOTAL // N_CORES
    nc = _get_nc(n_img)
    consts = _consts()
    preds = {"main_pred": main_pred, "ds1": ds1, "ds2": ds2, "ds3": ds3}
    in_maps = []
    for core in range(N_CORES):
        sl = slice(core * n_img, (core + 1) * n_img)
        m = {nm: np.ascontiguousarray(
                np.asarray(v).reshape(N_IMG_TOTAL, H, W)[sl]).astype(np.float32)
             for nm, v in preds.items()}
        m["target"] = np.ascontiguousarray(
            np.asarray(target).reshape(N_IMG_TOTAL, H, W)[sl]).astype(np.float32)
        m.update(consts)
        in_maps.append(m)

    res = run_bass_kernel_spmd(nc, in_maps, list(range(N_CORES)), trace=_trace)
    outs = [r["out"] for r in res.results]
    total = combine_partials(outs, N_IMG_TOTAL * H * W)
    if _trace:
        kernel._last_result = res
    return np.asarray(total, dtype=np.float32)
